# revision 9
# baseline (speedup 1.0000x reference)
"""LookAheadMask kernel for Trainium2 — in-place, pure-write, 32-row tiling.

out[b, r, c] = 1.0 if c > r else x[b, r, c], for x of shape (8, 4096, 4096) f32.

Sharding: batch dim across 8 NeuronCores (data parallel, no communication).

The output aliases the input buffer (lowering_input_output_aliases={0: 0}
through the BIR-lowering/NKI path), so everything at/below the diagonal
never moves, and the kernel writes ONLY the strictly-upper triangle:
33.55 MB per core, ZERO reads.

Empirical model (v1/v4 traces, all 16 DMA engines, all 8 cores live):
the engine pool is byte-bound at ~424-440 GB/s per core for big
descriptors (~26.5 B/ns/engine + ~6 ns/desc); small descriptors cost
~10-25 ns of an engine slot, and — critically — small-descriptor DMAs
queued on a HWDGE ring throttle to ~18 ns/desc under byte contention,
stalling everything behind them in that ring's FIFO (v4: ACT sat idle
for 37 us while SP drained staircase descs then streamed alone).

Decomposition (all access patterns 2D):
  - Bulk: 127 rectangles [32 rows x (4064-32g)] covering cols >=
    rowgroup+32 — 33.29 MB, 4064 descriptors (avg 8.2 KiB), split into
    exactly-equal byte halves between the SP and ACT HWDGE rings
    (g % 4 in {0,3} -> SP, {1,2} -> ACT), so both rings run byte-dense
    start to finish.
  - The strict upper triangles of the 128 diagonal 32x32 blocks are
    ragged per-row writes: row m of each group covers cols [m+1, 32),
    one dma_start per m = 0..29 (128 descriptors of 31-m f32 each), all
    on the gpsimd SWDGE queue — a third queue, so the ~4k tiny
    descriptors never block a HWDGE ring (SWDGE also coalesces strided
    rows into ~4 KiB 2D descriptors). The m=30 single-element group
    needs the non-contiguous-DMA escape hatch, which SWDGE failed to
    load with, so it rides at the head of the SP ring instead.
  - Two-stage ones memset ([:, :2048] then the rest) so the ragged
    writes and the narrow half of the bulk start at ~2 us.
"""

import numpy as np

S = 4096
G = 32  # bulk row-group height
NG = S // G  # 128 row groups; group 127 has no bulk rectangle
ONES_W = 4064  # widest bulk rectangle (group 0)
M1_W = 2048  # first-stage memset width; covers groups g >= 63 and ragged
N_CORES = 8

SP_BULK = [g for g in range(NG - 1) if g % 4 in (0, 3)]  # 63 starts
ACT_BULK = [g for g in range(NG - 1) if g % 4 in (1, 2)]  # 64 starts

_cached = None


def _build():
    from concourse import bass, mybir

    nc = bass.Bass(target_bir_lowering=True, enable_partition_id=False)
    x = nc.dram_tensor("x", [S, S], mybir.dt.float32, kind="ExternalInput")
    out = nc.dram_tensor("out", [S, S], mybir.dt.float32, kind="ExternalOutput")

    N_WRITES = len(SP_BULK) + len(ACT_BULK) + 31  # 158 dma_starts

    def bulk(eng, groups, ones, dsem, narrow=None):
        for g in groups:
            if narrow is not None and narrow != (g >= 63):
                continue
            r0 = g * G
            w = S - r0 - G
            eng.dma_start(
                out=out[r0 : r0 + G, r0 + G : S], in_=ones[:G, :w]
            ).then_inc(dsem, 16)

    def ragged(eng, ms, ones, dsem):
        # Row m of each 32-row group covers cols [m+1, 32) of the
        # group-diagonal 32x32 block: 128 descriptors of L = 31-m f32.
        for m in ms:
            L = 31 - m
            with nc.allow_non_contiguous_dma(
                reason="last ragged group writes isolated single f32 cells"
            ):
                eng.dma_start(
                    out=bass.AP(
                        out, 1 + m * (S + 1), [[G * (S + 1), NG], [1, L]]
                    ),
                    in_=ones[:, :L],
                ).then_inc(dsem, 16)

    with (
        nc.Block() as block,
        nc.semaphore("dsem") as dsem,  # all output-write DMA completions
        nc.semaphore("m1") as m1,  # ones[:, :2048] memset done
        nc.semaphore("msem") as msem,  # full ones memset done
        nc.sbuf_tensor("ones", [128, ONES_W], mybir.dt.float32) as ones,
    ):

        @block.vector
        def _(vector: bass.BassVectorEngine):
            vector.memset(ones[:, :M1_W], 1.0).then_inc(m1, 1)
            vector.memset(ones[:, M1_W:], 1.0).then_inc(msem, 1)

        @block.sync
        def _(sync: bass.BassEngine):
            sync.wait_ge(m1, 1)
            ragged(sync, [30], ones, dsem)
            bulk(sync, SP_BULK, ones, dsem, narrow=True)
            sync.wait_ge(msem, 1)
            bulk(sync, SP_BULK, ones, dsem, narrow=False)
            sync.wait_ge(dsem, 16 * N_WRITES)

        @block.scalar
        def _(scalar: bass.BassEngine):
            scalar.wait_ge(m1, 1)
            bulk(scalar, ACT_BULK, ones, dsem, narrow=True)
            scalar.wait_ge(msem, 1)
            bulk(scalar, ACT_BULK, ones, dsem, narrow=False)

        @block.gpsimd
        def _(gpsimd: bass.BassGpSimd):
            gpsimd.wait_ge(m1, 1)
            ragged(gpsimd, range(30), ones, dsem)

    nc.finalize()
    return nc


def _make_runner():
    """Compile-once runner: jit(shard_map(_body)) over 8 cores with the
    output aliased to the (donated) input — mirrors
    bass2jax.run_bass_via_pjrt, plus lowering_input_output_aliases."""
    global _cached
    if _cached is not None:
        return _cached

    import jax
    from jax.sharding import Mesh, PartitionSpec
    from jax.experimental.shard_map import shard_map
    from concourse import bass2jax

    bass2jax.install_neuronx_cc_hook()
    nc = _build()

    def _body(xg):
        outs = bass2jax._bass_exec_p.bind(
            xg,
            out_avals=(jax.core.ShapedArray((S, S), np.float32),),
            in_names=("x",),
            out_names=("out",),
            lowering_input_output_aliases=((0, 0),),
            sim_require_finite=True,
            sim_require_nnan=True,
            nc=nc,
        )
        return tuple(outs)

    devices = jax.devices()[:N_CORES]
    assert len(devices) == N_CORES, f"need {N_CORES} devices, have {len(devices)}"
    mesh = Mesh(np.asarray(devices), ("core",))
    sharded = jax.jit(
        shard_map(
            _body,
            mesh=mesh,
            in_specs=(PartitionSpec("core"),),
            out_specs=(PartitionSpec("core"),),
            check_rep=False,
        ),
        donate_argnums=(0,),
        keep_unused=True,
    )
    _cached = (nc, sharded)
    return _cached


class _Result:
    def __init__(self, exec_time_ns=None, mean_exec_time_ns=None):
        self.exec_time_ns = exec_time_ns
        self.mean_exec_time_ns = mean_exec_time_ns


def _run(x_full: np.ndarray, trace: bool = False):
    nc, sharded = _make_runner()
    x_full = np.asarray(x_full, dtype=np.float32)
    xg = np.ascontiguousarray(x_full.reshape(N_CORES * S, S))

    if not trace:
        out = sharded(xg)[0]
        return np.asarray(out).reshape(N_CORES, S, S), _Result()

    # Trace path (test.py only): NTFF profile around the execution, then the
    # same gauge/perfetto pipeline run_bass_kernel_spmd uses under axon.
    import glob
    import os
    import tempfile

    from antenv.axon_hooks import get_axon_ntff_profile_hook
    from concourse import bass_utils as BU

    neff_dir = tempfile.mkdtemp()
    hook = get_axon_ntff_profile_hook()
    with hook(neff_dir, [0]):
        out = np.asarray(sharded(xg)[0])

    ntffs = glob.glob(os.path.join(neff_dir, "*_body*.ntff"))
    if not ntffs:
        return out.reshape(N_CORES, S, S), _Result()

    sharepath = BU.upload_artifacts(neff_dir)
    profile = BU.gauge.profiler.Profile(
        profile_path=BU.FishPath(neff_dir),
        kernel_dev_mode=True,
        profile_on_exit=False,
        bass_kernel=nc.m,
        offline_processing=True,
        fname="*_body*",
        annotate_hlo=False,
        metadata={"artifacts_path": sharepath},
    )
    perf = BU._process_ntff_profile(
        profile,
        neff_dir,
        nc,
        list(range(N_CORES)),
        None,
        False,
        {},
        trace_events=False,
    )
    return out.reshape(N_CORES, S, S), _Result(
        perf.exec_time_ns, perf.mean_exec_time_ns
    )


def kernel(x: np.ndarray) -> np.ndarray:
    out, _ = _run(x, trace=False)
    return out


# revision 13
# speedup vs baseline: 1.7930x; 1.7930x over previous
"""LookAheadMask kernel for Trainium2 — in-place, pure-write, 32-row tiling.

out[b, r, c] = 1.0 if c > r else x[b, r, c], for x of shape (8, 4096, 4096) f32.

Sharding: batch dim across 8 NeuronCores (data parallel, no communication).

The output aliases the input buffer (lowering_input_output_aliases={0: 0}
through the BIR-lowering/NKI path), so everything at/below the diagonal
never moves, and the kernel writes ONLY the strictly-upper triangle:
33.55 MB per core, ZERO reads.

Empirical model (v1/v4 traces, all 16 DMA engines, all 8 cores live):
the engine pool is byte-bound at ~424-440 GB/s per core for big
descriptors (~26.5 B/ns/engine + ~6 ns/desc); small descriptors cost
~10-25 ns of an engine slot, and — critically — small-descriptor DMAs
queued on a HWDGE ring throttle to ~18 ns/desc under byte contention,
stalling everything behind them in that ring's FIFO (v4: ACT sat idle
for 37 us while SP drained staircase descs then streamed alone).

Decomposition (all access patterns 2D):
  - Bulk: 127 rectangles [32 rows x (4064-32g)] covering cols >=
    rowgroup+32 — 33.29 MB, 4064 descriptors (avg 8.2 KiB), split into
    exactly-equal byte halves between the SP and ACT HWDGE rings
    (g % 4 in {0,3} -> SP, {1,2} -> ACT), so both rings run byte-dense
    start to finish.
  - The strict upper triangles of the 128 diagonal 32x32 blocks are
    ragged per-row writes: row m of each group covers cols [m+1, 32),
    one dma_start per m = 0..29 (128 descriptors of 31-m f32 each), all
    on the gpsimd SWDGE queue — a third queue, so the ~4k tiny
    descriptors never block a HWDGE ring (SWDGE also coalesces strided
    rows into ~4 KiB 2D descriptors). The m=30 single-element group
    needs the non-contiguous-DMA escape hatch, which SWDGE failed to
    load with, so it rides at the head of the SP ring instead.
  - Two-stage ones memset ([:, :2048] then the rest) so the ragged
    writes and the narrow half of the bulk start at ~2 us.
"""

import numpy as np

S = 4096
P = 128  # bulk row-block height
NG = S // 32  # 128 ragged row groups
ONES_W = 3968  # widest bulk rectangle (block 0)
M1_W = 2048  # first-stage memset width; covers narrow bulk + all fine work
N_CORES = 8

# Bulk blocks b=0..30 write out[128b:128b+128, 128b+128:4096] (width
# 3968-128b). This byte-split is exactly 50/50 (pairs (4k,4k+3)/(4k+1,4k+2)).
SP_BULK = [0, 3, 4, 7, 8, 11, 12, 15, 16, 19, 20, 23, 24, 27, 28]
ACT_BULK = [1, 2, 5, 6, 9, 10, 13, 14, 17, 18, 21, 22, 25, 26, 29, 30]

_cached = None


def _build():
    from concourse import bass, mybir

    nc = bass.Bass(target_bir_lowering=True, enable_partition_id=False)
    x = nc.dram_tensor("x", [S, S], mybir.dt.float32, kind="ExternalInput")
    out = nc.dram_tensor("out", [S, S], mybir.dt.float32, kind="ExternalOutput")

    N_WRITES = len(SP_BULK) + len(ACT_BULK) + 2 + 31  # 64 dma_starts

    def bulk(eng, blocks, ones, dsem, narrow=None):
        for b in blocks:
            if narrow is not None and narrow != (b >= 15):
                continue
            r0 = b * P
            w = S - r0 - P
            eng.dma_start(
                out=out[r0 : r0 + P, r0 + P : S], in_=ones[:, :w]
            ).then_inc(dsem, 16)

    def level(eng, l, ones, dsem):
        # Binary staircase level l of the strict upper triangles of the 32
        # diagonal 128x128 blocks: n = 32<<l rects of h = 64>>l at stride
        # (128>>l)*(S+1). Partition r of rect k sources ones[r, k*h:k*h+h]
        # (n*h == 2048, inside the first-stage memset).
        s = 128 >> l
        h = s >> 1
        n = 32 << l
        eng.dma_start(
            out=bass.AP(out, h, [[S, h], [s * (S + 1), n], [1, h]]),
            in_=bass.AP(ones, 0, [[ONES_W, h], [h, n], [1, h]]),
        ).then_inc(dsem, 16)

    def ragged(eng, ms, ones, dsem):
        # Row m of each 32-row group covers cols [m+1, 32) of the
        # group-diagonal 32x32 block: 128 descriptors of L = 31-m f32.
        for m in ms:
            L = 31 - m
            with nc.allow_non_contiguous_dma(
                reason="last ragged group writes isolated single f32 cells"
            ):
                eng.dma_start(
                    out=bass.AP(
                        out, 1 + m * (S + 1), [[32 * (S + 1), NG], [1, L]]
                    ),
                    in_=ones[:, :L],
                ).then_inc(dsem, 16)

    with (
        nc.Block() as block,
        nc.semaphore("dsem") as dsem,  # all output-write DMA completions
        nc.semaphore("m1") as m1,  # ones[:, :2048] memset done
        nc.semaphore("msem") as msem,  # full ones memset done
        nc.sbuf_tensor("ones", [128, ONES_W], mybir.dt.float32) as ones,
    ):

        @block.vector
        def _(vector: bass.BassVectorEngine):
            vector.memset(ones[:, :M1_W], 1.0).then_inc(m1, 1)
            vector.memset(ones[:, M1_W:], 1.0).then_inc(msem, 1)

        @block.sync
        def _(sync: bass.BassEngine):
            sync.wait_ge(m1, 1)
            ragged(sync, [30], ones, dsem)
            bulk(sync, SP_BULK, ones, dsem, narrow=True)
            sync.wait_ge(msem, 1)
            bulk(sync, SP_BULK, ones, dsem, narrow=False)
            sync.wait_ge(dsem, 16 * N_WRITES)

        @block.scalar
        def _(scalar: bass.BassEngine):
            scalar.wait_ge(m1, 1)
            bulk(scalar, ACT_BULK, ones, dsem, narrow=True)
            scalar.wait_ge(msem, 1)
            bulk(scalar, ACT_BULK, ones, dsem, narrow=False)

        @block.gpsimd
        def _(gpsimd: bass.BassGpSimd):
            gpsimd.wait_ge(m1, 1)
            level(gpsimd, 0, ones, dsem)
            level(gpsimd, 1, ones, dsem)
            ragged(gpsimd, range(30), ones, dsem)

    nc.finalize()
    return nc


def _make_runner():
    """Compile-once runner: jit(shard_map(_body)) over 8 cores with the
    output aliased to the (donated) input — mirrors
    bass2jax.run_bass_via_pjrt, plus lowering_input_output_aliases."""
    global _cached
    if _cached is not None:
        return _cached

    import jax
    from jax.sharding import Mesh, PartitionSpec
    from jax.experimental.shard_map import shard_map
    from concourse import bass2jax

    bass2jax.install_neuronx_cc_hook()
    nc = _build()

    def _body(xg):
        outs = bass2jax._bass_exec_p.bind(
            xg,
            out_avals=(jax.core.ShapedArray((S, S), np.float32),),
            in_names=("x",),
            out_names=("out",),
            lowering_input_output_aliases=((0, 0),),
            sim_require_finite=True,
            sim_require_nnan=True,
            nc=nc,
        )
        return tuple(outs)

    devices = jax.devices()[:N_CORES]
    assert len(devices) == N_CORES, f"need {N_CORES} devices, have {len(devices)}"
    mesh = Mesh(np.asarray(devices), ("core",))
    sharded = jax.jit(
        shard_map(
            _body,
            mesh=mesh,
            in_specs=(PartitionSpec("core"),),
            out_specs=(PartitionSpec("core"),),
            check_rep=False,
        ),
        donate_argnums=(0,),
        keep_unused=True,
    )
    _cached = (nc, sharded)
    return _cached


class _Result:
    def __init__(self, exec_time_ns=None, mean_exec_time_ns=None):
        self.exec_time_ns = exec_time_ns
        self.mean_exec_time_ns = mean_exec_time_ns


def _run(x_full: np.ndarray, trace: bool = False):
    nc, sharded = _make_runner()
    x_full = np.asarray(x_full, dtype=np.float32)
    xg = np.ascontiguousarray(x_full.reshape(N_CORES * S, S))

    if not trace:
        out = sharded(xg)[0]
        return np.asarray(out).reshape(N_CORES, S, S), _Result()

    # Trace path (test.py only): NTFF profile around the execution, then the
    # same gauge/perfetto pipeline run_bass_kernel_spmd uses under axon.
    import glob
    import os
    import tempfile

    from antenv.axon_hooks import get_axon_ntff_profile_hook
    from concourse import bass_utils as BU

    neff_dir = tempfile.mkdtemp()
    hook = get_axon_ntff_profile_hook()
    with hook(neff_dir, [0]):
        out = np.asarray(sharded(xg)[0])

    ntffs = glob.glob(os.path.join(neff_dir, "*_body*.ntff"))
    if not ntffs:
        return out.reshape(N_CORES, S, S), _Result()

    sharepath = BU.upload_artifacts(neff_dir)
    profile = BU.gauge.profiler.Profile(
        profile_path=BU.FishPath(neff_dir),
        kernel_dev_mode=True,
        profile_on_exit=False,
        bass_kernel=nc.m,
        offline_processing=True,
        fname="*_body*",
        annotate_hlo=False,
        metadata={"artifacts_path": sharepath},
    )
    perf = BU._process_ntff_profile(
        profile,
        neff_dir,
        nc,
        list(range(N_CORES)),
        None,
        False,
        {},
        trace_events=False,
    )
    return out.reshape(N_CORES, S, S), _Result(
        perf.exec_time_ns, perf.mean_exec_time_ns
    )


def kernel(x: np.ndarray) -> np.ndarray:
    out, _ = _run(x, trace=False)
    return out
